# revision 5
# baseline (speedup 1.0000x reference)
"""MoCo hard-example-mining loss (topk_masking) on 8 Trainium2 NeuronCores.

Strategy (sharding_hint: shard queue along K):
  The reference computes dist = euclid(feat_q, queue_eff.T) [N=512, K=65536],
  then masked max (hard positive) / min (hard negative) per row, then a
  scalar soft-margin loss.  After the enqueue step, queue_eff columns are:
    - cols [0, 512):  feat_k.T with labels = targets   (the "special" block)
    - cols [512, 64K): original L2-normalized queue columns, labels = 0
  For the zero-label region the mask is row-constant, and ||y_j||^2 == 1,
  so per row only max_j / min_j of p_ij = <feat_q_i, y_j> over that region
  is needed.  That is a 512x512x65024 matmul + row max/min — the entire
  device workload.  The 512-column special block and the final scalar loss
  are computed exactly on the host in float64 (trivial cost).

  Device: the 65024 zero-label columns are padded to 65536 with duplicate
  columns (harmless for max/min) and sharded 8192 per core.  Each core:
    P = feat_q @ slab  (bf16 in, fp32 psum; 256 matmuls of [128x128]x[128x512])
    rowmax/rowmin of P  (ACT copies psum->sbuf bf16, DVE running max/min)
  returning two [512] vectors; host reduces across cores.
"""

import sys
import types
import numpy as np
import ml_dtypes

N, DIM, K, B = 512, 512, 65536, 512
NCORES = 8
KZ = K - B            # zero-label columns
CPC = K // NCORES     # padded columns per core (8192)
NT = CPC // 512       # 512-wide column tiles per core
BIG = 9999999.0

LAST_RESULTS = None   # BassKernelResults of the most recent device run
_NC_CACHE = {}


def _install_axon_hooks_shim():
    """antenv.axon_hooks is absent on this image; bass_utils imports it when
    NTFF tracing is requested.  Provide the tiny get/set module and register
    the ctypes-based NTFF hook so trace=True / BASS_TRACE=1 works."""
    try:
        import antenv  # noqa: F401
    except ImportError:
        return
    if "antenv.axon_hooks" in sys.modules:
        return
    mod = types.ModuleType("antenv.axon_hooks")
    mod._hook = None

    def set_axon_ntff_profile_hook(h):
        mod._hook = h

    def get_axon_ntff_profile_hook():
        return mod._hook

    mod.set_axon_ntff_profile_hook = set_axon_ntff_profile_hook
    mod.get_axon_ntff_profile_hook = get_axon_ntff_profile_hook
    sys.modules["antenv.axon_hooks"] = mod
    sys.modules["antenv"].axon_hooks = mod
    try:
        from trn_agent_boot.trn_boot import _ntff_profile_via_ctypes

        mod._hook = _ntff_profile_via_ctypes("/opt/axon/libaxon_pjrt.so")
    except Exception:
        pass


def _build_nc():
    """Build + compile the per-core Bass program (identical on all cores)."""
    import concourse.bacc as bacc
    import concourse.mybir as mybir
    from concourse.tile import TileContext

    bf16 = mybir.dt.bfloat16
    f32 = mybir.dt.float32

    nc = bacc.Bacc("TRN2", debug=False, target_bir_lowering=False)
    qT = nc.dram_tensor("qT", [DIM, N], bf16, kind="ExternalInput")
    slab = nc.dram_tensor("slab", [DIM, CPC], bf16, kind="ExternalInput")
    # o[:, 0:4] = per-m rowmax, o[:, 4:8] = per-m rowmin (row index = m*128+p)
    o = nc.dram_tensor("o", [128, 8], f32, kind="ExternalOutput")

    # DRAM views with the 4 contraction chunks as a middle axis, so one DMA
    # fills a whole [128, 4, 512] SBUF tile (row d = kk*128 + p).
    qT_v = qT.ap().rearrange("(k p) m -> p k m", p=128)
    slab_v = slab.ap().rearrange("(k p) c -> p k c", p=128)

    with TileContext(nc) as tc:
        with (
            tc.tile_pool(name="qpool", bufs=1) as qpool,
            tc.tile_pool(name="spool", bufs=6) as spool,
            tc.tile_pool(name="bpool", bufs=6) as bpool,
            tc.tile_pool(name="rpool", bufs=1) as rpool,
            tc.tile_pool(name="opool", bufs=1) as opool,
            tc.tile_pool(name="pspool", bufs=8, space="PSUM") as pspool,
        ):
            # stage qT: chunk 0 alone first so the first LDWEIGHTS can start
            # as early as possible, then chunks 1-3
            qt = qpool.tile([128, 4, N], bf16, name="qt")
            nc.sync.dma_start(out=qt[:, 0:1, :], in_=qT_v[:, 0:1, :])
            nc.sync.dma_start(out=qt[:, 1:4, :], in_=qT_v[:, 1:4, :])

            # two-phase running tiles: A covers n in [0,8), B covers [8,16);
            # A's final reduce overlaps the phase-B matmul stream
            rmxA = rpool.tile([128, 4, 512], bf16, name="rmxA")
            rmnA = rpool.tile([128, 4, 512], bf16, name="rmnA")
            rmxB = rpool.tile([128, 4, 512], bf16, name="rmxB")
            rmnB = rpool.tile([128, 4, 512], bf16, name="rmnB")
            oA = opool.tile([128, 8], f32, name="oA")
            osb = opool.tile([128, 8], f32, name="osb")

            for n in range(NT):
                st = spool.tile([128, 4, 512], bf16, name="st", tag="st")
                nc.sync.dma_start(out=st, in_=slab_v[:, :, n * 512 : (n + 1) * 512])
                rmx = rmxA if n < 8 else rmxB
                rmn = rmnA if n < 8 else rmnB
                for m in range(4):
                    ps = pspool.tile([128, 512], f32, name="ps", tag="ps")
                    for kk in range(4):
                        nc.tensor.matmul(
                            ps,
                            qt[:, kk, m * 128 : (m + 1) * 128],
                            st[:, kk, :],
                            start=(kk == 0),
                            stop=(kk == 3),
                        )
                    bt = bpool.tile([128, 512], bf16, name="bt", tag="bt")
                    nc.scalar.copy(bt, ps)  # ACT: psum fp32 -> sbuf bf16
                    if n in (0, 8):
                        nc.vector.tensor_copy(rmx[:, m, :], bt)
                        nc.vector.tensor_copy(rmn[:, m, :], bt)
                    else:
                        nc.vector.tensor_max(rmx[:, m, :], rmx[:, m, :], bt)
                        nc.vector.tensor_tensor(
                            rmn[:, m, :], rmn[:, m, :], bt, op=mybir.AluOpType.min
                        )
                if n == 8:
                    # phase-A reduction, overlapped with phase-B matmuls
                    nc.vector.tensor_reduce(
                        oA[:, 0:4], rmxA, axis=mybir.AxisListType.X,
                        op=mybir.AluOpType.max,
                    )
                    nc.vector.tensor_reduce(
                        oA[:, 4:8], rmnA, axis=mybir.AxisListType.X,
                        op=mybir.AluOpType.min,
                    )

            nc.vector.tensor_reduce(
                osb[:, 0:4], rmxB, axis=mybir.AxisListType.X, op=mybir.AluOpType.max
            )
            nc.vector.tensor_reduce(
                osb[:, 4:8], rmnB, axis=mybir.AxisListType.X, op=mybir.AluOpType.min
            )
            nc.vector.tensor_max(osb[:, 0:4], osb[:, 0:4], oA[:, 0:4])
            nc.vector.tensor_tensor(
                osb[:, 4:8], osb[:, 4:8], oA[:, 4:8], op=mybir.AluOpType.min
            )
            nc.sync.dma_start(out=o.ap(), in_=osb)

    nc.compile()
    return nc


def _get_nc():
    if "nc" not in _NC_CACHE:
        _install_axon_hooks_shim()
        _NC_CACHE["nc"] = _build_nc()
    return _NC_CACHE["nc"]


def _host_reference(feat_q, feat_k, targets, queue, queue_label):
    """Exact numpy fallback (float64) — used only if input assumptions
    (zero labels / normalized columns outside the enqueue block) fail."""
    fq = feat_q.astype(np.float64)
    fk = feat_k.astype(np.float64)
    t = targets.astype(np.int64)
    q = queue.astype(np.float64).copy()
    ql = queue_label.astype(np.int64).copy()
    q[:, : fk.shape[0]] = fk.T
    ql[: fk.shape[0]] = t
    xx = (fq * fq).sum(1)[:, None]
    yy = (q * q).sum(0)[None, :]
    sq = xx + yy - 2.0 * (fq @ q)
    dist = np.sqrt(np.clip(sq, 1e-12, None))
    is_pos = t[:, None] == ql[None, :]
    dist_ap = np.max(dist - BIG * (~is_pos), axis=1)
    dist_an = np.min(dist + BIG * is_pos, axis=1)
    return _loss(dist_ap, dist_an)


def _loss(dist_ap, dist_an):
    diff = dist_an - dist_ap
    loss_soft = np.mean(np.logaddexp(0.0, -diff))
    if np.isinf(loss_soft):
        return np.float32(np.mean(np.maximum(dist_ap - dist_an + 0.3, 0.0)))
    return np.float32(loss_soft)


def kernel(feat_q, feat_k, targets, queue, queue_label):
    feat_q = np.asarray(feat_q, dtype=np.float32)
    feat_k = np.asarray(feat_k, dtype=np.float32)
    targets = np.asarray(targets)
    queue = np.asarray(queue, dtype=np.float32)
    queue_label = np.asarray(queue_label)

    t = targets.astype(np.int64)
    Z = queue[:, B:]  # zero-label region, untouched by the enqueue

    # Guards for the two structural assumptions this split relies on.
    ok = not np.any(queue_label != 0)
    if ok:
        sample = np.linspace(0, KZ - 1, 512, dtype=np.int64)
        yy_s = np.einsum("ij,ij->j", Z[:, sample], Z[:, sample], dtype=np.float64)
        ok = bool(np.max(np.abs(yy_s - 1.0)) < 1e-3)
    if not ok:
        return _host_reference(feat_q, feat_k, targets, queue, queue_label)

    # ---- device part: rowmax/rowmin of feat_q @ Z over the zero-label region
    qT16 = np.ascontiguousarray(feat_q.T).astype(ml_dtypes.bfloat16)
    Z16 = Z.astype(ml_dtypes.bfloat16)
    in_maps = []
    for c in range(NCORES):
        lo = c * CPC
        hi = min((c + 1) * CPC, KZ)
        sl = np.empty((DIM, CPC), dtype=ml_dtypes.bfloat16)
        sl[:, : hi - lo] = Z16[:, lo:hi]
        if hi - lo < CPC:  # pad the tail core with duplicate columns
            sl[:, hi - lo :] = Z16[:, : CPC - (hi - lo)]
        in_maps.append({"qT": qT16, "slab": sl})

    from concourse import bass_utils

    nc = _get_nc()
    res = bass_utils.run_bass_kernel_spmd(nc, in_maps, core_ids=list(range(NCORES)))
    global LAST_RESULTS
    LAST_RESULTS = res

    pmax = np.full(N, -np.inf)
    pmin = np.full(N, np.inf)
    for c in range(NCORES):
        oc = np.asarray(res.results[c]["o"], dtype=np.float64)  # [128, 8]
        pmax = np.maximum(pmax, oc[:, 0:4].T.reshape(N))  # row (m*128+p) <- [p, m]
        pmin = np.minimum(pmin, oc[:, 4:8].T.reshape(N))

    # ---- host part: special 512-column block, exact in float64
    fq = feat_q.astype(np.float64)
    fk = feat_k.astype(np.float64)
    xx = (fq * fq).sum(1)
    kk_ = (fk * fk).sum(1)
    G = fq @ fk.T
    sqB = xx[:, None] + kk_[None, :] - 2.0 * G
    distB = np.sqrt(np.clip(sqB, 1e-12, None))
    maskB = t[:, None] == t[None, :]
    apB = np.max(distB - BIG * (~maskB), axis=1)
    anB = np.min(distB + BIG * maskB, axis=1)

    # zero-label region: ||y_j||^2 == 1, mask is row-constant (targets_i == 0)
    d_zmax = np.sqrt(np.clip(xx + 1.0 - 2.0 * pmin, 1e-12, None))
    d_zmin = np.sqrt(np.clip(xx + 1.0 - 2.0 * pmax, 1e-12, None))
    ap_z = d_zmax - BIG * (t != 0)
    an_z = d_zmin + BIG * (t == 0)

    dist_ap = np.maximum(apB, ap_z)
    dist_an = np.minimum(anB, an_z)
    return _loss(dist_ap, dist_an)
